# revision 44
# baseline (speedup 1.0000x reference)
"""Multi-head attention (B=2, S=2048, H=2048, NH=16) on 8 TRN2 NeuronCores.

Sharding: tensor-parallel over heads — 2 heads per core. Each core computes
q/k/v projections for its heads, per-head attention, and a partial output
projection (its heads' columns of Wo); the host sums the 8 partials.

Schedule (v3): two instruction streams interleaved quantum-by-quantum by a
host-side scheduler so the Tensor engine never idles behind softmax exp:
  - attention stream (Scalar-exp heavy): per (b,qh,h) block, score matmuls
    [128,512] -> exp (Scalar, bias=-SHIFT, scale=1/sqrt(hd)) -> AV matmuls,
    software-pipelined depth-3 (AV for tile i-3 behind scores of tile i);
    softmax denominator = 2-level DVE bf16 tree (to quads) + 4 accumulating
    ones-matmuls per 512-query span, reciprocal+scale on DVE. The tree depth
    balances DVE vs PE load in the attention+o-proj window.
  - projection stream (PE heavy): qkv chains per 512-token hT quarter
    (V token-major FD=256, Q/K feature-major FD=512), then o-proj in
    512-wide chunks; PSUM->SBUF staging copies run on DVE while attention
    is live (Act is exp-saturated there) and split Act/DVE in the tail;
    one out-DMA per 128-token row.
PSUM: 8 banks as 1-bank [128,512] f32 tiles: sc x3, av x2, work x2, den x1;
o-proj borrows the freed sc/av slots after attention is fully issued.
All input DMAs ride ONE queue in strict first-need order (descriptor
generation is a serial ~630ns/DMA resource and the transfer FIFO is served
in generation order): wv | hT-q0 interleaved, wq_lo, biases, wq_hi, wk,
hT-q1..q3, woT, hT-q4..q7.
"""

import sys

sys.path.insert(0, "/opt/trn_rl_repo")

from contextlib import ExitStack

import ml_dtypes
import numpy as np

import concourse.bass as bass
import concourse.tile as tile
from concourse import bacc, mybir
from concourse.bass_utils import run_bass_kernel_spmd

B, S, H, NH = 2, 2048, 2048, 16
HD = H // NH          # 128
N_CORES = 8
HPC = NH // N_CORES   # heads per core = 2
HDC = HPC * HD        # head-dims per core = 256
T = B * S             # 4096 tokens
FC = H // 128         # 16 feature chunks
TC = S // 128         # 16 token tiles per batch
QTOK = 512            # tokens per hT quarter-tile
NQ = S // QTOK        # 4 quarters per batch
SHIFT = 4.0           # fixed exp shift (softmax-invariant, overflow guard)

BF16 = mybir.dt.bfloat16
F32 = mybir.dt.float32
EXP = mybir.ActivationFunctionType.Exp

_CACHE = {}


def build_program(out_dtype=BF16):
    nc = bacc.Bacc(
        "TRN2", target_bir_lowering=False, debug=False, num_devices=N_CORES
    )
    hT = nc.dram_tensor("hT", [H, T], BF16, kind="ExternalInput").ap()
    wqT = nc.dram_tensor("wqT", [H, HDC], BF16, kind="ExternalInput").ap()
    wkT = nc.dram_tensor("wkT", [H, HDC], BF16, kind="ExternalInput").ap()
    wvT = nc.dram_tensor("wvT", [H, HDC], BF16, kind="ExternalInput").ap()
    woT = nc.dram_tensor("woT", [HDC, H], BF16, kind="ExternalInput").ap()
    bq = nc.dram_tensor("bq", [HDC], F32, kind="ExternalInput").ap()
    bk = nc.dram_tensor("bk", [HDC], F32, kind="ExternalInput").ap()
    bv = nc.dram_tensor("bv", [1, HDC], F32, kind="ExternalInput").ap()
    out = nc.dram_tensor("out", [T, H], out_dtype, kind="ExternalOutput").ap()

    with tile.TileContext(nc) as tc:
        _kernel(tc, out, hT, wqT, wkT, wvT, woT, bq, bk, bv)
    nc.compile()
    return nc


def _kernel(tc, out, hT, wqT, wkT, wvT, woT, bq, bk, bv):
    nc = tc.nc
    scale = 1.0 / float(np.sqrt(HD))
    ctx = ExitStack()
    with ctx:
        singles = ctx.enter_context(tc.tile_pool(name="singles", bufs=1))
        persist = ctx.enter_context(tc.tile_pool(name="persist", bufs=1))
        psum = ctx.enter_context(tc.tile_pool(name="psum", bufs=1, space="PSUM"))
        ht_pool = ctx.enter_context(tc.tile_pool(name="ht", bufs=4))
        pt_pool = ctx.enter_context(tc.tile_pool(name="pt", bufs=10))
        tree_pool = ctx.enter_context(tc.tile_pool(name="treeA", bufs=2))
        recip_pool = ctx.enter_context(tc.tile_pool(name="recip", bufs=2))
        o_sb_pool = ctx.enter_context(tc.tile_pool(name="o_sb", bufs=4))

        # ---- constants ----
        ones = singles.tile([128, 128], BF16)
        nc.vector.memset(ones, 1.0)
        neg_shift = singles.tile([128, 1], F32)
        nc.vector.memset(neg_shift, -SHIFT)
        scrap = singles.tile([128, 512], BF16)
        nc.vector.memset(scrap, 0.0)

        # ---- weight / bias DMAs (scalar-engine HWDGE queue) ----
        # The DMA transfer FIFO is served roughly in descriptor-generation
        # order, so issue strictly by first-need: wv (first consumer, 4-fc
        # chunks for a fast start), small biases, then wq/wk; woT goes LAST
        # (first o-proj is ~150us in, and issuing it early delays the hT
        # quarter streams behind it).
        w_sb = {}
        for name in ("v", "q", "k"):
            w_sb[name] = singles.tile(
                [128, FC, HDC], BF16, tag=f"w{name}", name=f"w{name}")

        def w_dma(name, ap, step, lo=0, hi=FC):
            re = ap.rearrange("(c p) m -> p c m", p=128)
            for c0 in range(lo, hi, step):
                nc.scalar.dma_start(
                    out=w_sb[name][:, c0 : c0 + step, :],
                    in_=re[:, c0 : c0 + step, :],
                )

        # ---- hT quarter tiles (4-slot ring) ----
        hT_re = hT.rearrange("(c p) t -> p c t", p=128)
        ht_tiles = [
            ht_pool.tile([128, FC, QTOK], BF16, tag="ht", name=f"ht{qi}")
            for qi in range(B * NQ)
        ]

        def ht_dma(qi, chunks, lo=0, engine=None):
            t0 = qi * QTOK
            c0 = lo
            for s in chunks:
                (engine or nc.scalar).dma_start(
                    out=ht_tiles[qi][:, c0 : c0 + s, :],
                    in_=hT_re[:, c0 : c0 + s, t0 : t0 + QTOK],
                )
                c0 += s

        # Single DMA queue, strict first-need order: the transfer FIFO is
        # served in descriptor-generation order, and with two queues the
        # interleave is a race. Arrival(350GB/s) vs need(PE):
        #   wv 3.3/4, q0 9.3/4-11, wq_lo 9.8/11, bv2 10.5/11.5, wq_hi 11.5/13,
        #   wk 14.4/18 (q-chains run before k-chains), q1 19.8/21, q2 25/28...
        # head of the queue: interleave wv / ht-q0 chunks so the first
        # v-chain matmul has both operands ~3us sooner
        ht_dma(0, (4, 4, 4, 4), engine=nc.sync)  # parallel gen at t=0
        w_dma("v", wvT, 4)
        w_dma("q", wqT, 8, hi=8)     # fc 0-7
        bv2 = singles.tile([128, 2, HDC], F32)
        nc.scalar.dma_start(
            out=bv2,
            in_=bass.AP(tensor=bv.tensor, offset=bv.offset,
                        ap=[[0, 128], [0, 2], [1, HDC]]),
        )
        bq_sb = singles.tile([128, HPC], F32)
        nc.scalar.dma_start(out=bq_sb, in_=bq.rearrange("(h p) -> p h", p=128))
        bk_sb = singles.tile([128, HPC], F32)
        nc.scalar.dma_start(out=bk_sb, in_=bk.rearrange("(h p) -> p h", p=128))
        w_dma("q", wqT, 8, lo=8)     # fc 8-15
        w_dma("k", wkT, 8)
        ht_dma(1, (8, 8))
        ht_dma(2, (8, 8))
        ht_dma(3, (8, 8))
        woT_sb = singles.tile([128, HPC, H], BF16)
        woT_re = woT.rearrange("(h p) o -> p h o", p=128)
        for h in range(HPC):
            nc.scalar.dma_start(out=woT_sb[:, h, :], in_=woT_re[:, h, :])
        for qi in range(NQ, B * NQ):
            ht_dma(qi, (8, 8))

        # ---- persistent activations ----
        qt_sb = [[persist.tile([128, S], BF16, tag=f"qt{b}{h}", name=f"qt{b}{h}")
                  for h in range(HPC)] for b in range(B)]
        kt_sb = [[persist.tile([128, S], BF16, tag=f"kt{b}{h}", name=f"kt{b}{h}")
                  for h in range(HPC)] for b in range(B)]
        v_sb = [persist.tile([128, TC, HDC], BF16, tag=f"v{b}", name=f"v{b}")
                for b in range(B)]
        aoT_sb = [[persist.tile([128, S], BF16, tag=f"ao{b}{h}", name=f"ao{b}{h}")
                   for h in range(HPC)] for b in range(B)]

        # PE pstate warmup: the tensor engine needs ~3us of sustained work
        # to reach full clock, and the first real matmul cannot start before
        # its operands land (~4.4us). Bridge the DMA wait with matmuls on
        # constants into the den PSUM bank (unused until the first attention
        # block) so real work starts at full speed.
        for wi in range(10):
            wps = psum.tile([128, 512], F32, tag="den", bufs=1,
                            name=f"warm{wi}")
            nc.tensor.matmul(wps, ones, scrap, start=True, stop=True)

        # done-flags set by generators as work is *issued* (issue order ==
        # per-engine execution order, so gating on issue is sound)
        done = set()

        # ---- projection stream: qkv for batch b, one quarter at a time ----
        def qkv_quanta(b):
            for q in range(NQ):
                ht = ht_tiles[b * NQ + q]
                # V chains: token-major, 2 chains of [128, 2, HDC]
                for vc in range(2):
                    ps = psum.tile([128, 2, HDC], F32, tag="work", bufs=2,
                                   name=f"vps{b}{q}{vc}")
                    # sub OUTER, fc inner: both subs share one 2KB PSUM
                    # zero-region, and a start=True matmul marks the whole
                    # region pending-zero — interleaving starts mid-chain
                    # would discard the other sub's partial sum
                    for sub in range(2):
                        tok0 = (vc * 2 + sub) * 128
                        for fc in range(FC):
                            nc.tensor.matmul(
                                ps[:, sub, :],
                                ht[:, fc, tok0 : tok0 + 128],
                                w_sb["v"][:, fc, :],
                                start=(fc == 0),
                                stop=(fc == FC - 1),
                            )
                            if fc % 4 == 3:
                                yield 436
                    tt0 = q * 4 + vc * 2
                    nc.vector.tensor_add(v_sb[b][:, tt0 : tt0 + 2, :], ps, bv2)
                # Q/K chains: feature-major [128, 512] spans; all Q
                # chains before K chains so wk can arrive later in the
                # input DMA stream
                for name0 in ("q", "k"):
                    for h in range(HPC):
                        name, dst, bias = (
                            (name0, qt_sb[b][h], bq_sb) if name0 == "q"
                            else (name0, kt_sb[b][h], bk_sb))
                        ps = psum.tile([128, QTOK], F32, tag="work", bufs=2,
                                       name=f"qkps{b}{q}{h}{name}")
                        for fc in range(FC):
                            nc.tensor.matmul(
                                ps,
                                w_sb[name][:, fc, h * HD : (h + 1) * HD],
                                ht[:, fc, :],
                                start=(fc == 0),
                                stop=(fc == FC - 1),
                            )
                            if fc % 4 == 3:
                                yield 864
                        nc.vector.tensor_scalar_add(
                            dst[:, q * QTOK : (q + 1) * QTOK],
                            ps, bias[:, h : h + 1],
                        )
            done.add(("qkv", b))

        # ---- attention stream: blocks (b, qh, h); software-pipelined ----
        def att_quanta(b):
            for qh in range(2):
                for h in range(HPC):
                    q0 = qh * 1024
                    avs = [psum.tile([128, 512], F32, tag="av", bufs=2,
                                     name=f"av{b}{h}{qh}{n}") for n in range(2)]
                    prev_pt = [None, None]
                    stacks = [[], []]  # (level, tile) binary-counter trees

                    def tree_push(span, t, tag_idx):
                        st = stacks[span]
                        lv = 0
                        while st and st[-1][0] == lv and lv < 2:
                            _, other = st.pop()
                            nt = tree_pool.tile(
                                [128, 512], BF16, tag=f"lv{lv + 1}",
                                bufs=(8 if lv + 1 == 2 else 4),
                                name=f"tr{b}{h}{qh}{span}{tag_idx}{lv}")
                            nc.vector.tensor_add(nt, other, t)
                            t = nt
                            lv += 1
                        st.append((lv, t))

                    # depth-2 software pipeline: AV for tile i-2 runs behind
                    # the score matmuls of tile i, so AV never waits on exp
                    pend = []  # (tcx, [ptA, ptB]) not yet fed to AV
                    for tcx in range(TC):
                        pts = []
                        for span in range(2):
                            sc = psum.tile([128, 512], F32, tag="sc", bufs=3,
                                           name=f"sc{b}{h}{qh}{tcx}{span}")
                            nc.tensor.matmul(
                                sc,
                                kt_sb[b][h][:, tcx * 128 : (tcx + 1) * 128],
                                qt_sb[b][h][:, q0 + span * 512 : q0 + (span + 1) * 512],
                                start=True,
                                stop=True,
                            )
                            pt = pt_pool.tile([128, 512], BF16, tag="pt",
                                              name=f"pt{b}{h}{qh}{tcx}{span}")
                            nc.scalar.activation(pt, sc, EXP,
                                                 bias=neg_shift, scale=scale)
                            pts.append(pt)
                        pend.append((tcx, pts))
                        if len(pend) > 3:
                            pv, ppts = pend.pop(0)
                            for span in range(2):
                                nc.tensor.matmul(
                                    avs[span],
                                    v_sb[b][:, pv, h * HD : (h + 1) * HD],
                                    ppts[span],
                                    start=(pv == 0),
                                    stop=False,
                                )
                                tree_push(span, ppts[span], pv)
                        yield 864
                    for pv, ppts in pend:
                        for span in range(2):
                            nc.tensor.matmul(
                                avs[span],
                                v_sb[b][:, pv, h * HD : (h + 1) * HD],
                                ppts[span],
                                start=(pv == 0),
                                stop=(pv == TC - 1),
                            )
                            tree_push(span, ppts[span], pv)
                        yield 432
                    # denominator + normalize, one 512-span at a time
                    for span in range(2):
                        quads = [t for _, t in stacks[span]]
                        assert len(quads) == 4
                        den = psum.tile([128, 512], F32, tag="den", bufs=1,
                                        name=f"den{b}{h}{qh}{span}")
                        for qi2, quad in enumerate(quads):
                            nc.tensor.matmul(den, ones, quad,
                                             start=(qi2 == 0),
                                             stop=(qi2 == len(quads) - 1))
                        recip = recip_pool.tile([128, 512], F32, tag="recip",
                                                name=f"r{b}{h}{qh}{span}")
                        nc.vector.reciprocal_approx_fast(recip, den)
                        nc.vector.tensor_mul(
                            aoT_sb[b][h][:, q0 + span * 512 : q0 + (span + 1) * 512],
                            avs[span], recip)
                    yield 432
                    done.add(("att", b, qh, h))

        # ---- o-proj stream: 128-token tiles, [128,1024] psum chunks ----
        o_copy_rr = [0]

        # PSUM->SBUF staging copies alternate between Act and DVE (GPSIMD
        # cannot read PSUM on TRN2)
        copy_engines = (
            lambda d, s: nc.scalar.copy(d, s),
            lambda d, s: nc.vector.tensor_copy(d, s),
        )

        def oproj_quanta(b, tts, borrow=False):
            tts = list(tts)
            # 512-wide chunks, each in a 1-bank PSUM tile; in borrow mode
            # (attention fully issued) also rotate over the freed sc/av slots
            # so the solo tail never stalls on a copy drain. One staging tile
            # and ONE out-DMA per 128-token row: descriptor generation is a
            # serial ~630ns/DMA resource, so fewer, bigger DMAs win.
            for tt in tts:
                row0 = b * S + tt * 128
                o_tile = o_sb_pool.tile([128, 2048], out.dtype, tag="o",
                                        name=f"ot{b}{tt}")
                for chunk in range(4):
                    if borrow:
                        tag, bufs = (("work", 2), ("sc", 3), ("av", 2))[
                            o_copy_rr[0] % 3]
                    else:
                        tag, bufs = "work", 2
                    ps = psum.tile([128, 512], F32, tag=tag, bufs=bufs,
                                   name=f"ops{b}{tt}{chunk}")
                    o0 = chunk * 512
                    for h in range(HPC):
                        nc.tensor.matmul(
                            ps,
                            aoT_sb[b][h][:, tt * 128 : (tt + 1) * 128],
                            woT_sb[:, h, o0 : o0 + 512],
                            start=(h == 0),
                            stop=(h == HPC - 1),
                        )
                    # one engine per o_tile (alternating per tile): the
                    # out-DMA then depends on a single engine's in-order
                    # stream rather than cross-engine subtile assembly.
                    # Final row: both engines, one per 1024-half, and a DMA
                    # per half — halves the post-last-matmul drain.
                    # while attention is live, Act is exp-saturated: all
                    # copies go to DVE; in the borrow tail both engines share
                    last = borrow and tt == tts[-1]
                    eng = ((chunk // 2) % 2 if last else tt % 2) if borrow else 1
                    copy_engines[eng](o_tile[:, o0 : o0 + 512], ps)
                    o_copy_rr[0] += 1
                    if last and chunk % 2 == 1:
                        nc.sync.dma_start(
                            out=out[row0 : row0 + 128, o0 - 512 : o0 + 512],
                            in_=o_tile[:, o0 - 512 : o0 + 512],
                        )
                    yield 432
                if not (borrow and tt == tts[-1]):
                    nc.sync.dma_start(
                        out=out[row0 : row0 + 128, :],
                        in_=o_tile,
                    )

        # ---- scheduler: alternate att and proj streams by issued PE-ns ----
        att_stream = [att_quanta(0), att_quanta(1)]
        proj_stream = [
            (qkv_quanta(0), None),
            (qkv_quanta(1), None),
            (oproj_quanta(0, range(0, 8)), ("att", 0, 0, HPC - 1)),
            (oproj_quanta(0, range(8, TC)), ("att", 0, 1, HPC - 1)),
            (oproj_quanta(1, range(0, 8)), ("att", 1, 0, HPC - 1)),
            (oproj_quanta(1, range(8, TC), borrow=True), ("att", 1, 1, HPC - 1)),
        ]
        att_gate = {0: ("qkv", 0), 1: ("qkv", 1)}
        ai = pi = 0
        att_ns = proj_ns = 0
        while ai < len(att_stream) or pi < len(proj_stream):
            stepped = False
            # att step (when its batch's qkv has been issued)
            if ai < len(att_stream) and att_gate[ai] in done:
                try:
                    att_ns += next(att_stream[ai])
                    stepped = True
                except StopIteration:
                    ai += 1
                    continue
            # proj steps until caught up with att stream
            while pi < len(proj_stream):
                gen, gate = proj_stream[pi]
                if gate is not None and gate not in done:
                    break
                try:
                    proj_ns += next(gen)
                    stepped = True
                except StopIteration:
                    pi += 1
                    continue
                if ai < len(att_stream) and proj_ns * 5 >= att_ns * 7:
                    break
            if not stepped:
                # both streams blocked on gates that only att progress can
                # open -> advance att unconditionally (its gate must be open
                # by construction: qkv precedes att in proj_stream)
                raise RuntimeError("scheduler deadlock")


def kernel(hidden_state, Wq, bq, Wk, bk, Wv, bv, Wo, bo):
    bf16 = ml_dtypes.bfloat16
    h2 = np.asarray(hidden_state, dtype=np.float32).reshape(T, H)
    hT = np.ascontiguousarray(h2.T).astype(bf16)

    in_maps = []
    for c in range(N_CORES):
        r0 = c * HDC
        in_maps.append({
            "hT": hT,
            "wqT": np.ascontiguousarray(
                np.asarray(Wq, np.float32)[r0 : r0 + HDC, :].T).astype(bf16),
            "wkT": np.ascontiguousarray(
                np.asarray(Wk, np.float32)[r0 : r0 + HDC, :].T).astype(bf16),
            "wvT": np.ascontiguousarray(
                np.asarray(Wv, np.float32)[r0 : r0 + HDC, :].T).astype(bf16),
            "woT": np.ascontiguousarray(
                np.asarray(Wo, np.float32)[:, r0 : r0 + HDC].T).astype(bf16),
            "bq": np.asarray(bq, np.float32)[r0 : r0 + HDC].copy(),
            "bk": np.asarray(bk, np.float32)[r0 : r0 + HDC].copy(),
            "bv": np.asarray(bv, np.float32)[r0 : r0 + HDC].reshape(1, HDC).copy(),
        })

    if "nc" not in _CACHE:
        _CACHE["nc"] = build_program()
    nc = _CACHE["nc"]
    _CACHE["in_maps"] = in_maps

    res = run_bass_kernel_spmd(nc, in_maps, core_ids=list(range(N_CORES)))
    total = np.zeros((T, H), np.float32)
    for r in res.results:
        total += np.asarray(r["out"]).astype(np.float32)
    total += np.asarray(bo, np.float32)[None, :]
    return total.reshape(B, S, H)
